# revision 14
# baseline (speedup 1.0000x reference)
"""Multi-head attention with bias, distributed over 8 trn2 NeuronCores.

Reference computation (per batch b):
    q = (x @ Wq.T) * depth**-0.5 ; k = y @ Wk.T ; v = y @ Wv.T     (per-head split)
    out = softmax(q @ k.T + bias) @ v @ Wo.T

Sharding v2 (tensor-parallel over heads): 8 cores = 4 batches x 2
head-groups of 8 heads.  Core c handles batch b = c//2 and heads
(c%2)*8 .. +8, over the FULL query sequence.  Wq/Wk/Wv are column-split
(by head) and Wo row-split; each core returns a partial output
[S, D] and the HOST sums the two partials per batch (the "all-reduce").
No redundant k/v projection work, no device collectives.

Device-side layout (feature dim on partitions):
    qT/kT = W.T-projected activations [dg=512, S]; head pair t lives on
    partitions of tile t.  v natural [kk, head, 97] with layout
    [0]*31 | 1 | v(64) | 1  so that
      even heads: lhsT cols 32:97 -> psum rows 0:64 attn + row 64 denom
      odd  heads: lhsT cols 0:96, base_partition 32 -> psum row 63 denom
                  + rows 64:128 attn  (legal PE out bases are {0,32,64})
    logitsT[kk, i] = kT-slice.T @ qT-slice  (K=64)
    expw = exp(logitsT) * exp(bias).T      (exp(bias) precomputed on host)
    attnT(+denom row) = [v|1].T @ expw     (K=128, denom rides along)
    normalization: DVE reciprocal of the denom row, PE ones-matmul
    broadcasts it across 64 partitions, DVE multiply writes normalized
    attn straight into SBUF in the out-projection layout.  outT = Wo.T
    partial projection, DMA'd directly from PSUM (f32).
Host does: transposes, bf16 casts, exp(bias), scale fold into Wq,
and the final pairwise partial sum.
"""

import numpy as np
import ml_dtypes
from contextlib import ExitStack

import concourse.bass as bass
import concourse.mybir as mybir
import concourse.tile as tile
from concourse import bacc
from concourse.bass_utils import run_bass_kernel_spmd

# full-problem dims (hardcoded per spec)
B, S, D, H = 4, 2048, 1024, 16
DEPTH = D // H            # 64
P = 128
NCORES = 8
HG = H // 2               # heads per core = 8
DG = HG * DEPTH           # feature dims per core = 512

BF = mybir.dt.bfloat16
F32 = mybir.dt.float32
EXP = mybir.ActivationFunctionType.Exp

TRACE = False
PACK_LOGITS = False        # v3: tile_position row-packing of K=64 logits mms
last_exec_time_ns = None
last_results = None


def _attn_body(ctx, tc, io):
    nc = tc.nc
    S_ = S                 # kv/q seq len (full)
    NT = D // P            # 8 input-dim tiles
    KT = S_ // P           # 16 kk tiles
    NDT = DG // P          # 4 head-pair tiles
    CW = 512               # free-dim chunk width
    NICH = S_ // CW        # 4 query chunks
    xT, yT, ebT, wqT, wkT, wvT, woT, outT = (
        io[k] for k in ("xT", "yT", "ebT", "wqT", "wkT", "wvT", "woT", "outT"))

    # ---- persistent pools (live through the whole kernel) ----
    qpool = ctx.enter_context(tc.tile_pool(name="qpool", bufs=NDT))
    kpool = ctx.enter_context(tc.tile_pool(name="kpool", bufs=NDT))
    vpool = ctx.enter_context(tc.tile_pool(name="vpool", bufs=KT))
    wopool = ctx.enter_context(tc.tile_pool(name="wopool", bufs=NDT))
    anpool = ctx.enter_context(tc.tile_pool(name="anpool", bufs=NDT))
    onepool = ctx.enter_context(tc.tile_pool(name="onepool", bufs=1))

    q_sb = [qpool.tile([P, S_], BF, tag="qT", name=f"q{t}", bufs=NDT)
            for t in range(NDT)]
    k_sb = [kpool.tile([P, S_], BF, tag="kT", name=f"k{t}", bufs=NDT)
            for t in range(NDT)]
    v_sb = [vpool.tile([P, HG, 66], BF, tag="v66", name=f"v{c}", bufs=KT)
            for c in range(KT)]
    wo_sb = [wopool.tile([P, D], BF, tag="wo", name=f"wo{t}", bufs=NDT)
             for t in range(NDT)]
    an_sb = [anpool.tile([P, S_], BF, tag="an", name=f"an{t}", bufs=NDT)
             for t in range(NDT)]
    ones_sb = onepool.tile([1, 64], BF, tag="ones", name="ones", bufs=1)
    nc.vector.memset(ones_sb, 1.0)

    # ---- psum: plp 4 banks now; pap/pbc (4 banks) allocated after the
    # prologue's ppj pool (2 banks) closes so peak stays at 8 banks ----
    plp = ctx.enter_context(tc.tile_pool(name="plp", bufs=2, space="PSUM"))

    # ================= prologue: load + q/k/v projections ==============
    with tc.tile_pool(name="xpool", bufs=NT) as xpool, \
         tc.tile_pool(name="ypool", bufs=NT) as ypool, \
         tc.tile_pool(name="wqpool", bufs=NT) as wqpool, \
         tc.tile_pool(name="wkpool", bufs=NT) as wkpool, \
         tc.tile_pool(name="wvpool", bufs=NT) as wvpool, \
         tc.tile_pool(name="ppj", bufs=2, space="PSUM") as ppj:
        x_sb = [xpool.tile([P, S_], BF, tag="xT", name=f"x{t}", bufs=NT)
                for t in range(NT)]
        wq_sb = [wqpool.tile([P, DG], BF, tag="wq", name=f"wq{t}", bufs=NT)
                 for t in range(NT)]
        y_sb = [ypool.tile([P, S_], BF, tag="yT", name=f"y{t}", bufs=NT)
                for t in range(NT)]
        wk_sb = [wkpool.tile([P, DG], BF, tag="wk", name=f"wk{t}", bufs=NT)
                 for t in range(NT)]
        wv_sb = [wvpool.tile([P, DG], BF, tag="wv", name=f"wv{t}", bufs=NT)
                 for t in range(NT)]
        for t in range(NT):
            nc.sync.dma_start(out=x_sb[t], in_=xT[t * P:(t + 1) * P, :])
            nc.sync.dma_start(out=wq_sb[t], in_=wqT[t * P:(t + 1) * P, :])
        for t in range(NT):
            nc.sync.dma_start(out=y_sb[t], in_=yT[t * P:(t + 1) * P, :])
            nc.sync.dma_start(out=wk_sb[t], in_=wkT[t * P:(t + 1) * P, :])
            nc.sync.dma_start(out=wv_sb[t], in_=wvT[t * P:(t + 1) * P, :])
        for t in range(NDT):
            nc.gpsimd.dma_start(out=wo_sb[t], in_=woT[t * P:(t + 1) * P, :])

        # warm-up heartbeats: tiny matmuls chained to arriving input DMAs
        # keep the PE HAM activity window alive through the load phase
        jnk0 = plp.tile([P, 1024], F32, tag="pl", name="jnk0", bufs=2)
        for t in range(NT):
            nc.tensor.matmul(jnk0[0:1, 0:CW], lhsT=x_sb[t][0:1, 0:1],
                             rhs=x_sb[t][0:1, 0:CW], start=True, stop=True)
            nc.tensor.matmul(jnk0[0:1, 0:CW], lhsT=y_sb[t][0:1, 0:1],
                             rhs=y_sb[t][0:1, 0:CW], start=True, stop=True)

        # q projection: qT[512, S] tiles [128, S]
        for td in range(NDT):
            for n0 in range(0, S_, CW):
                ps = ppj.tile([P, CW], F32, tag="pj", name=f"pq{td}_{n0}",
                              bufs=2)
                for u in range(NT):
                    nc.tensor.matmul(ps,
                                     lhsT=wq_sb[u][:, td * P:(td + 1) * P],
                                     rhs=x_sb[u][:, n0:n0 + CW],
                                     start=(u == 0), stop=(u == NT - 1))
                nc.vector.tensor_copy(q_sb[td][:, n0:n0 + CW], ps)
        # k projection
        for td in range(NDT):
            for n0 in range(0, S_, CW):
                ps = ppj.tile([P, CW], F32, tag="pj", name=f"pk{td}_{n0}",
                              bufs=2)
                for u in range(NT):
                    nc.tensor.matmul(ps,
                                     lhsT=wk_sb[u][:, td * P:(td + 1) * P],
                                     rhs=y_sb[u][:, n0:n0 + CW],
                                     start=(u == 0), stop=(u == NT - 1))
                nc.vector.tensor_copy(k_sb[td][:, n0:n0 + CW], ps)
        # v projection: natural [kk, head, depth] with ones/zeros padding
        for c in range(KT):
            vt = v_sb[c]
            nc.vector.memset(vt[:, :, 64:65], 1.0)
            ps = ppj.tile([P, CW], F32, tag="pj", name=f"pv{c}", bufs=2)
            for u in range(NT):
                nc.tensor.matmul(ps,
                                 lhsT=y_sb[u][:, c * P:(c + 1) * P],
                                 rhs=wv_sb[u][:, 0:DG],
                                 start=(u == 0), stop=(u == NT - 1))
            nc.vector.tensor_copy(
                vt[:, :, 0:64], ps.rearrange("p (h d) -> p h d", d=DEPTH))

    # eb tiles stream in now (first use is ~immediately below; DMA of
    # tile c completes well before the c-loop consumes it)
    ebpool = ctx.enter_context(tc.tile_pool(name="ebpool", bufs=KT))
    eb_sb = [ebpool.tile([P, S_], BF, tag="eb", name=f"eb{c}", bufs=KT)
             for c in range(KT)]
    for c in range(KT):
        nc.sync.dma_start(out=eb_sb[c], in_=ebT[c * P:(c + 1) * P, :])

    ewpool = ctx.enter_context(tc.tile_pool(name="ewpool", bufs=3))
    ew2pool = ctx.enter_context(tc.tile_pool(name="ew2pool", bufs=3))
    recpool = ctx.enter_context(tc.tile_pool(name="recpool", bufs=2))
    pap = ctx.enter_context(tc.tile_pool(name="pap", bufs=4, space="PSUM"))

    # ================= main loop =================
    LOOK = 2               # c-loop software pipeline depth
    for ich in range(NICH):
        isl = slice(ich * CW, (ich + 1) * CW)
        for t in range(NDT):
            pa = pap.tile([P, CW], F32, tag="pattn", name=f"pa{t}_{ich}",
                          bufs=4)
            pb = pap.tile([P, CW], F32, tag="pattn", name=f"pb{t}_{ich}",
                          bufs=4)
            ew2s = []

            def emit_L(c):
                plt = plp.tile([P, 1024], F32, tag="pl",
                               name=f"pl{t}_{ich}_{c}", bufs=2)
                kw = dict(start=True, stop=True)
                nc.tensor.matmul(plt[:, 0:CW],
                                 lhsT=k_sb[t][0:64, c * P:(c + 1) * P],
                                 rhs=q_sb[t][0:64, isl],
                                 tile_position=(0, 0) if PACK_LOGITS else None,
                                 **kw)
                nc.tensor.matmul(plt[:, CW:2 * CW],
                                 lhsT=k_sb[t][64:128, c * P:(c + 1) * P],
                                 rhs=q_sb[t][64:128, isl],
                                 tile_position=(64, 0) if PACK_LOGITS else None,
                                 **kw)
                ew = ewpool.tile([P, 1024], BF, tag="ew",
                                 name=f"ew{t}_{ich}_{c}", bufs=3)
                nc.scalar.activation(ew, plt, EXP)
                ew2 = ew2pool.tile([P, 1024], BF, tag="ew2",
                                   name=f"ew2{t}_{ich}_{c}", bufs=3)
                nc.vector.tensor_mul(ew2[:, 0:CW], ew[:, 0:CW],
                                     eb_sb[c][:, isl])
                nc.vector.tensor_mul(ew2[:, CW:2 * CW], ew[:, CW:2 * CW],
                                     eb_sb[c][:, isl])
                ew2s.append(ew2)

            def emit_A(c):
                ew2 = ew2s[c]
                kw = dict(start=(c == 0), stop=(c == KT - 1))
                nc.tensor.matmul(pa[0:65, :],
                                 lhsT=v_sb[c][:, 2 * t, 0:65],
                                 rhs=ew2[:, 0:CW], **kw)
                nc.tensor.matmul(pb[0:65, :],
                                 lhsT=v_sb[c][:, 2 * t + 1, 0:65],
                                 rhs=ew2[:, CW:2 * CW], **kw)

            for c in range(KT):
                emit_L(c)
                if c >= LOOK:
                    emit_A(c - LOOK)
            for c in range(KT - LOOK, KT):
                emit_A(c)

            # ---- normalization: recip(denom) -> PE broadcast -> scale ----
            raf = recpool.tile([1, CW], F32, tag="raf", name=f"raf{t}{ich}",
                               bufs=2)
            rbf = recpool.tile([1, CW], F32, tag="rbf", name=f"rbf{t}{ich}",
                               bufs=2)
            dab = recpool.tile([1, 2 * CW], F32, tag="dab", name=f"dab{t}{ich}",
                               bufs=2)
            nc.vector.tensor_copy(dab[:, 0:CW], pa[64:65, :])
            nc.vector.tensor_copy(dab[:, CW:2 * CW], pb[64:65, :])
            nc.vector.reciprocal_approx_fast(raf, dab[:, 0:CW])
            nc.vector.reciprocal_approx_fast(rbf, dab[:, CW:2 * CW])
            rab = recpool.tile([1, CW], BF, tag="rab", name=f"rab{t}{ich}",
                               bufs=2)
            rbb = recpool.tile([1, CW], BF, tag="rbb", name=f"rbb{t}{ich}",
                               bufs=2)
            nc.vector.tensor_copy(rab, raf)
            nc.vector.tensor_copy(rbb, rbf)
            bc = plp.tile([P, 1024], F32, tag="pl", name=f"bc{t}_{ich}",
                          bufs=2)
            nc.tensor.matmul(bc[0:64, 0:CW], lhsT=ones_sb, rhs=rab,
                             start=True, stop=True)
            nc.tensor.matmul(bc[0:64, CW:2 * CW], lhsT=ones_sb, rhs=rbb,
                             start=True, stop=True)
            bcs = recpool.tile([64, 1024], BF, tag="bcs", name=f"bcs{t}{ich}",
                               bufs=2)
            nc.vector.tensor_copy(bcs, bc[0:64, :])
            nc.vector.tensor_mul(an_sb[t][0:64, isl], pa[0:64, :],
                                 bcs[:, 0:CW])
            anb = recpool.tile([64, CW], BF, tag="anb", name=f"anb{t}{ich}",
                               bufs=2)
            nc.vector.tensor_mul(anb, pb[0:64, :], bcs[:, CW:2 * CW])
            nc.sync.dma_start(out=an_sb[t][64:128, isl], in_=anb)

        # ---- output projection for this query chunk (partial over Wo) ----
        for m in range(NT):
            po = pap.tile([P, CW], F32, tag="pattn", name=f"po{m}_{ich}",
                          bufs=4)
            for kt in range(NDT):
                nc.tensor.matmul(po,
                                 lhsT=wo_sb[kt][:, m * P:(m + 1) * P],
                                 rhs=an_sb[kt][:, isl],
                                 start=(kt == 0), stop=(kt == NDT - 1))
            osb = ew2pool.tile([P, CW], F32, tag="osb", name=f"o{m}_{ich}",
                               bufs=2)
            nc.vector.tensor_copy(osb, po)
            nc.gpsimd.dma_start(out=outT[m * P:(m + 1) * P, isl], in_=osb)


def build_nc():
    nc = bacc.Bacc("TRN2", target_bir_lowering=False, debug=False)
    io = {
        "xT": nc.dram_tensor("xT", [D, S], BF, kind="ExternalInput").ap(),
        "yT": nc.dram_tensor("yT", [D, S], BF, kind="ExternalInput").ap(),
        "ebT": nc.dram_tensor("ebT", [S, S], BF, kind="ExternalInput").ap(),
        "wqT": nc.dram_tensor("wqT", [D, DG], BF, kind="ExternalInput").ap(),
        "wkT": nc.dram_tensor("wkT", [D, DG], BF, kind="ExternalInput").ap(),
        "wvT": nc.dram_tensor("wvT", [D, DG], BF, kind="ExternalInput").ap(),
        "woT": nc.dram_tensor("woT", [DG, D], BF, kind="ExternalInput").ap(),
        "outT": nc.dram_tensor("outT", [D, S], F32,
                               kind="ExternalOutput").ap(),
    }
    with tile.TileContext(nc) as tc:
        with ExitStack() as ctx:
            _attn_body(ctx, tc, io)
    nc.compile()
    return nc


_NC_CACHE = None


def kernel(x, y, bias, Wq, Wk, Wv, Wo):
    global _NC_CACHE, last_exec_time_ns, last_results
    x = np.asarray(x, np.float32)
    y = np.asarray(y, np.float32)
    bias = np.asarray(bias, np.float32)
    Wq, Wk, Wv, Wo = (np.asarray(w, np.float32) for w in (Wq, Wk, Wv, Wo))
    if _NC_CACHE is None:
        _NC_CACHE = build_nc()
    nc = _NC_CACHE

    bf = ml_dtypes.bfloat16
    scale = DEPTH ** -0.5
    eb = np.exp(bias[0, 0].astype(np.float32))
    ebT = np.ascontiguousarray(eb.T).astype(bf)
    xT_all = [np.ascontiguousarray(x[b].T).astype(bf) for b in range(B)]
    yT_all = [np.ascontiguousarray(y[b].T).astype(bf) for b in range(B)]
    wqT_g, wkT_g, wvT_g, woT_g = [], [], [], []
    for g in range(2):
        rows = slice(g * DG, (g + 1) * DG)
        wqT_g.append(np.ascontiguousarray(Wq[rows, :].T * scale).astype(bf))
        wkT_g.append(np.ascontiguousarray(Wk[rows, :].T).astype(bf))
        wvT_g.append(np.ascontiguousarray(Wv[rows, :].T).astype(bf))
        woT_g.append(np.ascontiguousarray(Wo[:, rows].T).astype(bf))

    in_maps = []
    for core in range(NCORES):
        b, g = divmod(core, 2)
        in_maps.append({
            "xT": xT_all[b], "yT": yT_all[b], "ebT": ebT,
            "wqT": wqT_g[g], "wkT": wkT_g[g], "wvT": wvT_g[g],
            "woT": woT_g[g],
        })

    res = run_bass_kernel_spmd(nc, in_maps, core_ids=list(range(NCORES)),
                               trace=TRACE)
    last_exec_time_ns = res.exec_time_ns
    last_results = res
    out = np.empty((B, S, D), np.float32)
    for b in range(B):
        out[b] = (res.results[2 * b]["outT"].T.astype(np.float32)
                  + res.results[2 * b + 1]["outT"].T.astype(np.float32))
    return out


# revision 18
# speedup vs baseline: 1.1554x; 1.1554x over previous
"""Multi-head attention with bias, distributed over 8 trn2 NeuronCores.

Reference computation (per batch b):
    q = (x @ Wq.T) * depth**-0.5 ; k = y @ Wk.T ; v = y @ Wv.T     (per-head split)
    out = softmax(q @ k.T + bias) @ v @ Wo.T

Sharding v2 (tensor-parallel over heads): 8 cores = 4 batches x 2
head-groups of 8 heads.  Core c handles batch b = c//2 and heads
(c%2)*8 .. +8, over the FULL query sequence.  Wq/Wk/Wv are column-split
(by head) and Wo row-split; each core returns a partial output
[S, D] and the HOST sums the two partials per batch (the "all-reduce").
No redundant k/v projection work, no device collectives.

Device-side layout (feature dim on partitions):
    qT/kT = W.T-projected activations [dg=512, S]; head pair t lives on
    partitions of tile t.  v natural [kk, head, 97] with layout
    [0]*31 | 1 | v(64) | 1  so that
      even heads: lhsT cols 32:97 -> psum rows 0:64 attn + row 64 denom
      odd  heads: lhsT cols 0:96, base_partition 32 -> psum row 63 denom
                  + rows 64:128 attn  (legal PE out bases are {0,32,64})
    logitsT[kk, i] = kT-slice.T @ qT-slice  (K=64)
    expw = exp(logitsT) * exp(bias).T      (exp(bias) precomputed on host)
    attnT(+denom row) = [v|1].T @ expw     (K=128, denom rides along)
    normalization: DVE reciprocal of the denom row, PE ones-matmul
    broadcasts it across 64 partitions, DVE multiply writes normalized
    attn straight into SBUF in the out-projection layout.  outT = Wo.T
    partial projection, DMA'd directly from PSUM (f32).
Host does: transposes, bf16 casts, exp(bias), scale fold into Wq,
and the final pairwise partial sum.
"""

import numpy as np
import ml_dtypes
from contextlib import ExitStack

import concourse.bass as bass
import concourse.mybir as mybir
import concourse.tile as tile
from concourse import bacc
from concourse.bass_utils import run_bass_kernel_spmd

# full-problem dims (hardcoded per spec)
B, S, D, H = 4, 2048, 1024, 16
DEPTH = D // H            # 64
P = 128
NCORES = 8
HG = H // 2               # heads per core = 8
DG = HG * DEPTH           # feature dims per core = 512

BF = mybir.dt.bfloat16
F32 = mybir.dt.float32
EXP = mybir.ActivationFunctionType.Exp

TRACE = False
PACK_LOGITS = False        # v3: tile_position row-packing of K=64 logits mms
last_exec_time_ns = None
last_results = None


def _attn_body(ctx, tc, io):
    nc = tc.nc
    S_ = S                 # kv/q seq len (full)
    NT = D // P            # 8 input-dim tiles
    KT = S_ // P           # 16 kk tiles
    NDT = DG // P          # 4 head-pair tiles
    CW = 512               # free-dim chunk width
    NICH = S_ // CW        # 4 query chunks
    xT, yT, ebT, wqT, wkT, wvT, woT, outT = (
        io[k] for k in ("xT", "yT", "ebT", "wqT", "wkT", "wvT", "woT", "outT"))

    # ---- persistent pools (live through the whole kernel) ----
    qpool = ctx.enter_context(tc.tile_pool(name="qpool", bufs=NDT))
    kpool = ctx.enter_context(tc.tile_pool(name="kpool", bufs=NDT))
    vpool = ctx.enter_context(tc.tile_pool(name="vpool", bufs=KT))
    wopool = ctx.enter_context(tc.tile_pool(name="wopool", bufs=NDT))
    anpool = ctx.enter_context(tc.tile_pool(name="anpool", bufs=NDT))
    onepool = ctx.enter_context(tc.tile_pool(name="onepool", bufs=1))

    q_sb = [qpool.tile([P, S_], BF, tag="qT", name=f"q{t}", bufs=NDT)
            for t in range(NDT)]
    k_sb = [kpool.tile([P, S_], BF, tag="kT", name=f"k{t}", bufs=NDT)
            for t in range(NDT)]
    v_sb = [vpool.tile([P, HG, 66], BF, tag="v66", name=f"v{c}", bufs=KT)
            for c in range(KT)]
    wo_sb = [wopool.tile([P, D], BF, tag="wo", name=f"wo{t}", bufs=NDT)
             for t in range(NDT)]
    an_sb = [anpool.tile([P, S_], BF, tag="an", name=f"an{t}", bufs=NDT)
             for t in range(NDT)]
    ones_sb = onepool.tile([1, 64], BF, tag="ones", name="ones", bufs=1)
    nc.vector.memset(ones_sb, 1.0)

    # ---- psum: plp 4 banks now; pap/pbc (4 banks) allocated after the
    # prologue's ppj pool (2 banks) closes so peak stays at 8 banks ----
    plp = ctx.enter_context(tc.tile_pool(name="plp", bufs=2, space="PSUM"))

    # ================= prologue: load + q/k/v projections ==============
    with tc.tile_pool(name="xpool", bufs=NT) as xpool, \
         tc.tile_pool(name="ypool", bufs=NT) as ypool, \
         tc.tile_pool(name="wqpool", bufs=NT) as wqpool, \
         tc.tile_pool(name="wkpool", bufs=NT) as wkpool, \
         tc.tile_pool(name="wvpool", bufs=NT) as wvpool, \
         tc.tile_pool(name="ppj", bufs=2, space="PSUM") as ppj:
        x_sb = [xpool.tile([P, S_], BF, tag="xT", name=f"x{t}", bufs=NT)
                for t in range(NT)]
        wq_sb = [wqpool.tile([P, DG], BF, tag="wq", name=f"wq{t}", bufs=NT)
                 for t in range(NT)]
        y_sb = [ypool.tile([P, S_], BF, tag="yT", name=f"y{t}", bufs=NT)
                for t in range(NT)]
        wk_sb = [wkpool.tile([P, DG], BF, tag="wk", name=f"wk{t}", bufs=NT)
                 for t in range(NT)]
        wv_sb = [wvpool.tile([P, DG], BF, tag="wv", name=f"wv{t}", bufs=NT)
                 for t in range(NT)]
        for t in range(NT):
            nc.sync.dma_start(out=x_sb[t], in_=xT[t * P:(t + 1) * P, :])
            nc.sync.dma_start(out=wq_sb[t], in_=wqT[t * P:(t + 1) * P, :])
        for t in range(NT):
            nc.sync.dma_start(out=y_sb[t], in_=yT[t * P:(t + 1) * P, :])
            nc.sync.dma_start(out=wk_sb[t], in_=wkT[t * P:(t + 1) * P, :])
            nc.sync.dma_start(out=wv_sb[t], in_=wvT[t * P:(t + 1) * P, :])
        for t in range(NDT):
            nc.gpsimd.dma_start(out=wo_sb[t], in_=woT[t * P:(t + 1) * P, :])

        # warm-up heartbeats: tiny matmuls chained to arriving input DMAs
        # keep the PE HAM activity window alive through the load phase
        jnk0 = plp.tile([P, 1024], F32, tag="pl", name="jnk0", bufs=2)
        for t in range(NT):
            nc.tensor.matmul(jnk0[0:1, 0:CW], lhsT=x_sb[t][0:1, 0:1],
                             rhs=x_sb[t][0:1, 0:CW], start=True, stop=True)
            nc.tensor.matmul(jnk0[0:1, 0:CW], lhsT=y_sb[t][0:1, 0:1],
                             rhs=y_sb[t][0:1, 0:CW], start=True, stop=True)

        # q projection: qT[512, S] tiles [128, S]
        for td in range(NDT):
            for n0 in range(0, S_, CW):
                ps = ppj.tile([P, CW], F32, tag="pj", name=f"pq{td}_{n0}",
                              bufs=2)
                for u in range(NT):
                    nc.tensor.matmul(ps,
                                     lhsT=wq_sb[u][:, td * P:(td + 1) * P],
                                     rhs=x_sb[u][:, n0:n0 + CW],
                                     start=(u == 0), stop=(u == NT - 1))
                nc.vector.tensor_copy(q_sb[td][:, n0:n0 + CW], ps)
        # k projection
        for td in range(NDT):
            for n0 in range(0, S_, CW):
                ps = ppj.tile([P, CW], F32, tag="pj", name=f"pk{td}_{n0}",
                              bufs=2)
                for u in range(NT):
                    nc.tensor.matmul(ps,
                                     lhsT=wk_sb[u][:, td * P:(td + 1) * P],
                                     rhs=y_sb[u][:, n0:n0 + CW],
                                     start=(u == 0), stop=(u == NT - 1))
                nc.vector.tensor_copy(k_sb[td][:, n0:n0 + CW], ps)
        # v projection: natural [kk, head, depth] with ones/zeros padding
        for c in range(KT):
            vt = v_sb[c]
            nc.vector.memset(vt[:, :, 64:65], 1.0)
            ps = ppj.tile([P, CW], F32, tag="pj", name=f"pv{c}", bufs=2)
            for u in range(NT):
                nc.tensor.matmul(ps,
                                 lhsT=y_sb[u][:, c * P:(c + 1) * P],
                                 rhs=wv_sb[u][:, 0:DG],
                                 start=(u == 0), stop=(u == NT - 1))
            nc.vector.tensor_copy(
                vt[:, :, 0:64], ps.rearrange("p (h d) -> p h d", d=DEPTH))

    # eb tiles stream in now (first use is ~immediately below; DMA of
    # tile c completes well before the c-loop consumes it)
    ebpool = ctx.enter_context(tc.tile_pool(name="ebpool", bufs=KT))
    eb_sb = [ebpool.tile([P, S_], BF, tag="eb", name=f"eb{c}", bufs=KT)
             for c in range(KT)]
    for c in range(KT):
        nc.sync.dma_start(out=eb_sb[c], in_=ebT[c * P:(c + 1) * P, :])

    ewpool = ctx.enter_context(tc.tile_pool(name="ewpool", bufs=3))
    ew2pool = ctx.enter_context(tc.tile_pool(name="ew2pool", bufs=3))
    recpool = ctx.enter_context(tc.tile_pool(name="recpool", bufs=2))
    dpool = ctx.enter_context(tc.tile_pool(name="dpool", bufs=2,
                                           space="DRAM"))
    pap = ctx.enter_context(tc.tile_pool(name="pap", bufs=4, space="PSUM"))

    # ================= main loop =================
    # Software-pipelined: normalization of pair (t) and the out-projection
    # of chunk (ich) are emitted INSIDE later pairs' c-loops so the PE
    # instruction stream never blocks on the DVE/DMA normalization chain.
    LOOK = 2               # c-loop attnV lookahead depth

    def emit_norm(t, ich, pa, pb):
        """Phase 1 of normalization: recip(denoms) + DMA partition-bcast."""
        isl = slice(ich * CW, (ich + 1) * CW)
        dab = recpool.tile([1, 2 * CW], F32, tag="dab", name=f"dab{t}{ich}",
                           bufs=2)
        nc.vector.tensor_copy(dab[:, 0:CW], pa[64:65, :])
        nc.vector.tensor_copy(dab[:, CW:2 * CW], pb[64:65, :])
        raf = recpool.tile([1, 2 * CW], F32, tag="raf", name=f"raf{t}{ich}",
                           bufs=2)
        nc.vector.reciprocal_approx_fast(raf, dab)
        rab = recpool.tile([1, 2 * CW], BF, tag="rab", name=f"rab{t}{ich}",
                           bufs=2)
        nc.vector.tensor_copy(rab, raf)
        rdr = dpool.tile([1, 2 * CW], BF, tag="rdr", name=f"rdr{t}{ich}",
                         bufs=2)
        nc.sync.dma_start(out=rdr, in_=rab)
        bcs = recpool.tile([64, 2 * CW], BF, tag="bcs", name=f"bcs{t}{ich}",
                           bufs=2)
        nc.sync.dma_start(out=bcs[:, 0:CW],
                          in_=rdr[0:1, 0:CW].partition_broadcast(64))
        nc.sync.dma_start(out=bcs[:, CW:2 * CW],
                          in_=rdr[0:1, CW:2 * CW].partition_broadcast(64))
        return bcs

    def emit_norm2(t, ich, pa, pb, bcs):
        """Phase 2: scale unnormalized attn into the out-proj layout."""
        isl = slice(ich * CW, (ich + 1) * CW)
        nc.vector.tensor_mul(an_sb[t][0:64, isl], pa[0:64, :],
                             bcs[:, 0:CW])
        anb = recpool.tile([64, CW], BF, tag="anb", name=f"anb{t}{ich}",
                           bufs=2)
        nc.vector.tensor_mul(anb, pb[0:64, :], bcs[:, CW:2 * CW])
        nc.sync.dma_start(out=an_sb[t][64:128, isl], in_=anb)

    def emit_opm(ich, m):
        """One m-tile of the partial output projection for chunk ich."""
        isl = slice(ich * CW, (ich + 1) * CW)
        po = pap.tile([P, CW], F32, tag="pattn", name=f"po{m}_{ich}",
                      bufs=4)
        for kt in range(NDT):
            nc.tensor.matmul(po,
                             lhsT=wo_sb[kt][:, m * P:(m + 1) * P],
                             rhs=an_sb[kt][:, isl],
                             start=(kt == 0), stop=(kt == NDT - 1))
        osb = ew2pool.tile([P, CW], F32, tag="osb", name=f"o{m}_{ich}",
                           bufs=2)
        nc.vector.tensor_copy(osb, po)
        nc.gpsimd.dma_start(out=outT[m * P:(m + 1) * P, isl], in_=osb)

    pending_norm = None    # (t, ich, pa, pb) awaiting norm emission
    pending_op = None      # ich awaiting out-projection emission

    for ich in range(NICH):
        isl = slice(ich * CW, (ich + 1) * CW)
        for t in range(NDT):
            pa = pap.tile([P, CW], F32, tag="pattn", name=f"pa{t}_{ich}",
                          bufs=4)
            pb = pap.tile([P, CW], F32, tag="pattn", name=f"pb{t}_{ich}",
                          bufs=4)
            ew2s = []

            def emit_L(c, t=t, isl=isl, ew2s=ew2s):
                plt = plp.tile([P, 1024], F32, tag="pl",
                               name=f"pl{t}_{c}", bufs=2)
                kw = dict(start=True, stop=True)
                nc.tensor.matmul(plt[:, 0:CW],
                                 lhsT=k_sb[t][0:64, c * P:(c + 1) * P],
                                 rhs=q_sb[t][0:64, isl],
                                 tile_position=(0, 0) if PACK_LOGITS else None,
                                 **kw)
                nc.tensor.matmul(plt[:, CW:2 * CW],
                                 lhsT=k_sb[t][64:128, c * P:(c + 1) * P],
                                 rhs=q_sb[t][64:128, isl],
                                 tile_position=(64, 0) if PACK_LOGITS else None,
                                 **kw)
                ew = ewpool.tile([P, 1024], BF, tag="ew",
                                 name=f"ew{t}_{c}", bufs=3)
                nc.scalar.activation(ew, plt, EXP)
                ew2 = ew2pool.tile([P, 1024], BF, tag="ew2",
                                   name=f"ew2{t}_{c}", bufs=3)
                nc.vector.tensor_mul(ew2[:, 0:CW], ew[:, 0:CW],
                                     eb_sb[c][:, isl])
                nc.vector.tensor_mul(ew2[:, CW:2 * CW], ew[:, CW:2 * CW],
                                     eb_sb[c][:, isl])
                ew2s.append(ew2)

            def emit_A(c, t=t, pa=pa, pb=pb, ew2s=ew2s):
                ew2 = ew2s[c]
                kw = dict(start=(c == 0), stop=(c == KT - 1))
                nc.tensor.matmul(pa[0:65, :],
                                 lhsT=v_sb[c][:, 2 * t, 0:65],
                                 rhs=ew2[:, 0:CW], **kw)
                nc.tensor.matmul(pb[0:65, :],
                                 lhsT=v_sb[c][:, 2 * t + 1, 0:65],
                                 rhs=ew2[:, CW:2 * CW], **kw)

            bcs_pend = None
            for c in range(KT):
                emit_L(c)
                if c >= LOOK:
                    emit_A(c - LOOK)
                # deferred work from the previous pair / chunk rides along
                if c == 1 and pending_norm is not None:
                    bcs_pend = emit_norm(*pending_norm)
                elif c == 3 and pending_norm is not None:
                    emit_norm2(*pending_norm, bcs_pend)
                    pending_norm = None
                elif 6 <= c < 14 and pending_op is not None:
                    emit_opm(pending_op, c - 6)
                    if c == 13:
                        pending_op = None
            for c in range(KT - LOOK, KT):
                emit_A(c)
            pending_norm = (t, ich, pa, pb)
        pending_op = ich

    # tail: last pair's normalization + last chunk's out-projection
    bcs_last = emit_norm(*pending_norm)
    emit_norm2(*pending_norm, bcs_last)
    for m in range(NT):
        emit_opm(pending_op, m)


def build_nc():
    nc = bacc.Bacc("TRN2", target_bir_lowering=False, debug=False)
    io = {
        "xT": nc.dram_tensor("xT", [D, S], BF, kind="ExternalInput").ap(),
        "yT": nc.dram_tensor("yT", [D, S], BF, kind="ExternalInput").ap(),
        "ebT": nc.dram_tensor("ebT", [S, S], BF, kind="ExternalInput").ap(),
        "wqT": nc.dram_tensor("wqT", [D, DG], BF, kind="ExternalInput").ap(),
        "wkT": nc.dram_tensor("wkT", [D, DG], BF, kind="ExternalInput").ap(),
        "wvT": nc.dram_tensor("wvT", [D, DG], BF, kind="ExternalInput").ap(),
        "woT": nc.dram_tensor("woT", [DG, D], BF, kind="ExternalInput").ap(),
        "outT": nc.dram_tensor("outT", [D, S], F32,
                               kind="ExternalOutput").ap(),
    }
    with tile.TileContext(nc) as tc:
        with ExitStack() as ctx:
            _attn_body(ctx, tc, io)
    nc.compile()
    return nc


_NC_CACHE = None


def kernel(x, y, bias, Wq, Wk, Wv, Wo):
    global _NC_CACHE, last_exec_time_ns, last_results
    x = np.asarray(x, np.float32)
    y = np.asarray(y, np.float32)
    bias = np.asarray(bias, np.float32)
    Wq, Wk, Wv, Wo = (np.asarray(w, np.float32) for w in (Wq, Wk, Wv, Wo))
    if _NC_CACHE is None:
        _NC_CACHE = build_nc()
    nc = _NC_CACHE

    bf = ml_dtypes.bfloat16
    scale = DEPTH ** -0.5
    eb = np.exp(bias[0, 0].astype(np.float32))
    ebT = np.ascontiguousarray(eb.T).astype(bf)
    xT_all = [np.ascontiguousarray(x[b].T).astype(bf) for b in range(B)]
    yT_all = [np.ascontiguousarray(y[b].T).astype(bf) for b in range(B)]
    wqT_g, wkT_g, wvT_g, woT_g = [], [], [], []
    for g in range(2):
        rows = slice(g * DG, (g + 1) * DG)
        wqT_g.append(np.ascontiguousarray(Wq[rows, :].T * scale).astype(bf))
        wkT_g.append(np.ascontiguousarray(Wk[rows, :].T).astype(bf))
        wvT_g.append(np.ascontiguousarray(Wv[rows, :].T).astype(bf))
        woT_g.append(np.ascontiguousarray(Wo[:, rows].T).astype(bf))

    in_maps = []
    for core in range(NCORES):
        b, g = divmod(core, 2)
        in_maps.append({
            "xT": xT_all[b], "yT": yT_all[b], "ebT": ebT,
            "wqT": wqT_g[g], "wkT": wkT_g[g], "wvT": wvT_g[g],
            "woT": woT_g[g],
        })

    res = run_bass_kernel_spmd(nc, in_maps, core_ids=list(range(NCORES)),
                               trace=TRACE)
    last_exec_time_ns = res.exec_time_ns
    last_results = res
    out = np.empty((B, S, D), np.float32)
    for b in range(B):
        out[b] = (res.results[2 * b]["outT"].T.astype(np.float32)
                  + res.results[2 * b + 1]["outT"].T.astype(np.float32))
    return out
